# revision 1
# baseline (speedup 1.0000x reference)
"""Trainium2 Bass kernel: GPT-2-style causal multi-head attention.

Problem: B=4, S=2048, D=1024, H=16 heads (head_dim 64), fp32.
  q/k/v = x @ W{q,k,v} + b{q,k,v}; causal softmax attention; out = attn_out @ Wo + bo.

Sharding (8 cores): tensor-parallel over heads - each core owns 2 heads
(128 feature dims). Wq/Wk/Wv column-sliced, Wo row-sliced per core. Each core
computes a partial o_proj output (transposed, [D, B*S]); the host sums the 8
partials, transposes, and adds bo.

Layout strategy on-chip: everything is kept transposed ([feature, seq]) so that
all matmul contractions have their contraction dim on SBUF partitions:
  x^T (via PE transpose) -> q^T/k^T/v^T = W^T x^T -> S^T = K^T^T... scores
  computed as S^T[k, q] tiles -> exp on ACT -> P^T -> out^T = V^T-ext @ P^T
  (with an appended ones column producing the softmax denominators) ->
  normalize -> o_proj out^T = Wo^T attnout^T.
"""

import sys
import os

sys.path.insert(0, "/opt/trn_rl_repo")

import numpy as np

import concourse.bass as bass
import concourse.bacc as bacc
import concourse.tile as tile
import concourse.mybir as mybir
from concourse.bass_utils import run_bass_kernel_spmd

F32 = mybir.dt.float32
F32R = mybir.dt.float32r

B, S, D, H = 4, 2048, 1024, 16
HD = D // H  # 64
N_CORES = 8
HPC = H // N_CORES  # heads per core = 2
J = HPC * HD  # per-core feature dims = 128
BS = B * S  # 8192
NB = S // 128  # 16 s-blocks per batch
NC = S // 512  # 4 chunks of 512 per batch

# fast (relaxed-precision) fp32 for the big matmuls; exact fp32 for transposes.
# fp32r operands must be produced pre-rounded, so every tile feeding an fp32r
# matmul is declared float32r and written by a rounding copy/activation.
MM_DT = F32R


def build_kernel():
    nc = bacc.Bacc(
        "TRN2", target_bir_lowering=False, debug=False, enable_asserts=False,
        num_devices=N_CORES,
    )

    x_d = nc.dram_tensor("x", [BS, D], F32, kind="ExternalInput").ap()
    wq_d = nc.dram_tensor("wq", [D, J], F32, kind="ExternalInput").ap()
    wk_d = nc.dram_tensor("wk", [D, J], F32, kind="ExternalInput").ap()
    wv_d = nc.dram_tensor("wv", [D, J], F32, kind="ExternalInput").ap()
    wo_d = nc.dram_tensor("wo", [J, D], F32, kind="ExternalInput").ap()
    bq_d = nc.dram_tensor("bq", [J], F32, kind="ExternalInput").ap()
    bk_d = nc.dram_tensor("bk", [J], F32, kind="ExternalInput").ap()
    bv_d = nc.dram_tensor("bv", [J], F32, kind="ExternalInput").ap()
    out_d = nc.dram_tensor("out_t", [D, BS], F32, kind="ExternalOutput").ap()

    with tile.TileContext(nc) as tc:
        _emit(tc, nc, x_d, wq_d, wk_d, wv_d, wo_d, bq_d, bk_d, bv_d, out_d)

    nc.compile()
    return nc


def _emit(tc, nc, x_d, wq_d, wk_d, wv_d, wo_d, bq_d, bk_d, bv_d, out_d):
    from contextlib import ExitStack

    ctx = ExitStack()
    with ctx:
        const = ctx.enter_context(tc.tile_pool(name="const", bufs=1))
        wpool = ctx.enter_context(tc.tile_pool(name="w", bufs=1))
        xpool = ctx.enter_context(tc.tile_pool(name="x", bufs=6))
        xtpool = ctx.enter_context(tc.tile_pool(name="xt", bufs=12))
        qkvpool = ctx.enter_context(tc.tile_pool(name="qkv", bufs=2))
        vepool = ctx.enter_context(tc.tile_pool(name="ve", bufs=4))
        ptpool = ctx.enter_context(tc.tile_pool(name="pt", bufs=3))
        aopool = ctx.enter_context(tc.tile_pool(name="ao", bufs=2))
        nrmpool = ctx.enter_context(tc.tile_pool(name="nrm", bufs=2))
        stgpool = ctx.enter_context(tc.tile_pool(name="stg", bufs=3))
        ps_st = ctx.enter_context(tc.tile_pool(name="ps_st", bufs=2, space="PSUM"))
        ps_acc = ctx.enter_context(tc.tile_pool(name="ps_acc", bufs=2, space="PSUM"))
        ps_mm = ctx.enter_context(tc.tile_pool(name="ps_mm", bufs=2, space="PSUM"))

        # --- constants ---------------------------------------------------
        # identity[p, f] = 1 if p == f else 0   (for PE transpose)
        ident = const.tile([128, 128], F32, tag="ident")
        nc.gpsimd.memset(ident[:], 1.0)
        nc.gpsimd.affine_select(
            ident[:], ident[:], pattern=[[1, 128]],
            compare_op=mybir.AluOpType.is_equal, fill=0.0,
            base=0, channel_multiplier=-1,
        )
        # fp32r copy of the identity for transposing fp32r tiles (v^T)
        ident_r = const.tile([128, 128], MM_DT, tag="ident_r")
        nc.vector.tensor_copy(ident_r[:], ident[:])
        # causal mask for diagonal 128x128 blocks of S^T[k, q]:
        # keep (1.0) where k <= q i.e. f - p >= 0
        mask_f = const.tile([128, 128], F32, tag="mask_f")
        nc.gpsimd.memset(mask_f[:], 1.0)
        nc.gpsimd.affine_select(
            mask_f[:], mask_f[:], pattern=[[1, 128]],
            compare_op=mybir.AluOpType.is_ge, fill=0.0,
            base=0, channel_multiplier=-1,
        )
        mask = const.tile([128, 128], MM_DT, tag="mask")
        nc.vector.tensor_copy(mask[:], mask_f[:])
        # fp32r ones column-vector group for the softmax-denominator columns
        ones_f = const.tile([128, 16], F32, tag="ones_f")
        nc.gpsimd.memset(ones_f[:], 1.0)
        ones16 = const.tile([128, 16], MM_DT, tag="ones16")
        nc.vector.tensor_copy(ones16[:], ones_f[:])
        # fp32r ones [128, 64] for the recip partition-broadcast matmul
        ones64f = const.tile([128, 64], F32, tag="ones64f")
        nc.gpsimd.memset(ones64f[:], 1.0)
        ones64 = const.tile([128, 64], MM_DT, tag="ones64")
        nc.vector.tensor_copy(ones64[:], ones64f[:])

        # --- weights -----------------------------------------------------
        # wq/wk/wv: [D, J] -> one [128, 1024] tile per projection (contraction
        # block ib at cols [128*ib, 128*ib+128)). DMA can't cast to fp32r, so
        # stage as fp32 then round with a DVE copy.
        w_tiles = {}
        for name, wd in (("q", wq_d), ("k", wk_d), ("v", wv_d)):
            stg = wpool.tile([128, D], F32, tag="wstg", name="wstg", bufs=2)
            for ib in range(8):
                nc.sync.dma_start(
                    stg[:, ib * 128:(ib + 1) * 128],
                    wd[ib * 128:(ib + 1) * 128, :])
            t = wpool.tile([128, D], MM_DT, tag=f"w{name}", name=f"w{name}")
            nc.vector.tensor_copy(t[:], stg[:])
            w_tiles[name] = t
        wo_stg = wpool.tile([J, D], F32, tag="wstg", name="wo_stg", bufs=2)
        nc.sync.dma_start(wo_stg[:], wo_d[:, :])
        wo_t = wpool.tile([J, D], MM_DT, tag="wo")
        nc.vector.tensor_copy(wo_t[:], wo_stg[:])

        bias = {}
        for name, bd in (("q", bq_d), ("k", bk_d), ("v", bv_d)):
            t = const.tile([J, 1], F32, tag=f"b{name}")
            nc.sync.dma_start(t[:], bd.rearrange("(p o) -> p o", o=1))
            bias[name] = t

        # --- per-batch pipeline -----------------------------------------
        for b in range(B):
            s0 = b * S  # row offset into x / out^T columns

            # projections: q^T/k^T/v^T [J=128, 2048] for this batch.
            # Per 512-wide chunk: load x, PE-transpose to x^T, then the three
            # projection matmuls consume (and release) the chunk's x^T tiles.
            proj = {
                name: qkvpool.tile([J, S], MM_DT, tag=f"{name}t", name=f"{name}t")
                for name in ("q", "k", "v")
            }
            for c in range(NC):
                x_t = []
                for si in range(4):
                    sb = 4 * c + si
                    t = xpool.tile([128, D], F32, tag="x", name="x")
                    nc.sync.dma_start(
                        t[:], x_d[s0 + sb * 128: s0 + (sb + 1) * 128, :])
                    x_t.append(t)
                xt = []
                for ib in range(8):
                    pst = ps_mm.tile([128, 512], F32, tag="ps_mm", name="pst")
                    for si in range(4):
                        nc.tensor.transpose(
                            pst[:, si * 128:(si + 1) * 128],
                            x_t[si][:, ib * 128:(ib + 1) * 128],
                            ident[:],
                        )
                    t = xtpool.tile([128, 512], MM_DT, tag="xt", name="xt")
                    nc.vector.tensor_copy(t[:], pst[:])
                    xt.append(t)
                for name in ("q", "k", "v"):
                    pacc = ps_mm.tile([128, 512], F32, tag="ps_mm", name="pacc")
                    for ib in range(8):
                        nc.tensor.matmul(
                            pacc[:],
                            w_tiles[name][:, ib * 128:(ib + 1) * 128],
                            xt[ib][:],
                            start=(ib == 0), stop=(ib == 7),
                        )
                    # copy PSUM -> SBUF with per-partition bias add (on ACT)
                    nc.scalar.activation(
                        proj[name][:, c * 512:(c + 1) * 512], pacc[:],
                        mybir.ActivationFunctionType.Identity,
                        bias=bias[name][:],
                    )
            qt, kt, vt = proj["q"], proj["k"], proj["v"]

            # V natural (per head, with ones column appended):
            # ve[h]: [128 k, 16*65], block kb at cols [65*kb, 65*kb+65),
            # col 65*kb+64 is the ones column (softmax denominator trick).
            ve = []
            for h in range(HPC):
                t = vepool.tile([128, NB * 65], MM_DT, tag="ve")
                # ones columns at 65*kb + 64 via one strided copy
                nc.vector.tensor_copy(
                    t[:].rearrange("p (nb c) -> p nb c", c=65)[:, :, 64:65],
                    ones16[:].rearrange("p (a o) -> p a o", o=1),
                )
                ve.append(t)
            for sb in range(NB):
                pst = ps_mm.tile([128, 512], F32, tag="ps_mm")
                nc.tensor.transpose(
                    pst[:, 0:128].bitcast(MM_DT),
                    vt[:, sb * 128:(sb + 1) * 128], ident_r[:],
                )
                for h in range(HPC):
                    nc.vector.tensor_copy(
                        ve[h][:, sb * 65: sb * 65 + 64],
                        pst[:, h * 64:(h + 1) * 64].bitcast(MM_DT),
                    )

            # attention for each head
            aot = aopool.tile([J, S], MM_DT, tag="aot")  # attnout^T, heads stacked
            for h in range(HPC):
                hp = slice(h * HD, (h + 1) * HD)  # partition range of this head
                for p in range(2):  # chunk-pair passes: chunks {2p, 2p+1}
                    acc = [
                        ps_acc.tile([128, 512], F32, tag="ps_acc", name="acc0"),
                        ps_acc.tile([128, 512], F32, tag="ps_acc", name="acc1"),
                    ]
                    n_kb = 8 * p + 8
                    for kb in range(n_kb):
                        lo = max(0, 128 * kb - 1024 * p)  # local col offset
                        st = ps_st.tile([128, 1024], F32, tag="ps_st")
                        for half in range(2):
                            hlo = max(lo, 512 * half)
                            hhi = 512 * (half + 1)
                            if hlo >= hhi:
                                continue
                            nc.tensor.matmul(
                                st[:, hlo:hhi],
                                kt[hp, kb * 128:(kb + 1) * 128],
                                qt[hp, 1024 * p + hlo: 1024 * p + hhi],
                                start=True, stop=True,
                            )
                        pt = ptpool.tile([128, 1024], MM_DT, tag="pt")
                        nc.scalar.activation(
                            pt[:, lo:1024], st[:, lo:1024],
                            mybir.ActivationFunctionType.Exp,
                            scale=0.125,
                        )
                        # diagonal block (only when it falls in this pass):
                        # mask the lower triangle
                        if 128 * kb - 1024 * p >= 0:
                            nc.vector.tensor_mul(
                                pt[:, lo:lo + 128], pt[:, lo:lo + 128], mask[:],
                            )
                        for half in range(2):
                            chunk = 2 * p + half
                            if kb > 4 * chunk + 3:
                                continue
                            hlo = max(lo, 512 * half)
                            hhi = 512 * (half + 1)
                            nc.tensor.matmul(
                                acc[half][0:65, hlo - 512 * half: 512],
                                ve[h][:, kb * 65: kb * 65 + 65],
                                pt[:, hlo:hhi],
                                start=(kb == 0), stop=(kb == 4 * chunk + 3),
                            )
                    # normalize: rows 0..63 = unnormalized out^T, row 64 = rowsum
                    for half in range(2):
                        chunk = 2 * p + half
                        rec = nrmpool.tile([128, 512], F32, tag="rec")
                        nc.vector.reciprocal(rec[64:65, :], acc[half][64:65, :])
                        rec_r = nrmpool.tile([128, 512], MM_DT, tag="rec_r")
                        nc.vector.tensor_copy(rec_r[64:65, :], rec[64:65, :])
                        # broadcast recip row to partitions 0..63 via ones-col
                        # matmul (gpsimd partition_broadcast is unreliable)
                        bcp = ps_mm.tile([64, 512], F32, tag="ps_mm", name="bcp")
                        nc.tensor.matmul(
                            bcp[:], ones64[64:65, :], rec_r[64:65, :],
                            start=True, stop=True,
                        )
                        bct = nrmpool.tile([128, 512], F32, tag="bct")
                        nc.vector.tensor_copy(bct[0:64, :], bcp[:])
                        if h == 0:
                            nc.vector.tensor_mul(
                                aot[0:64, chunk * 512:(chunk + 1) * 512],
                                acc[half][0:64, :], bct[0:64, :],
                            )
                        else:
                            tmp = nrmpool.tile([64, 512], MM_DT, tag="tmp")
                            nc.vector.tensor_mul(
                                tmp[:], acc[half][0:64, :], bct[0:64, :],
                            )
                            # partition shift 0-63 -> 64-127 via SBUF->SBUF DMA
                            nc.sync.dma_start(
                                aot[64:128, chunk * 512:(chunk + 1) * 512], tmp[:],
                            )

            # o_proj: out^T[o, s] partial = Wo_slice^T @ attnout^T
            for ob in range(8):
                stg = stgpool.tile([128, S], F32, tag="stg")
                for c in range(NC):
                    pst = ps_mm.tile([128, 512], F32, tag="ps_mm")
                    nc.tensor.matmul(
                        pst[:],
                        wo_t[:, ob * 128:(ob + 1) * 128],
                        aot[:, c * 512:(c + 1) * 512],
                        start=True, stop=True,
                    )
                    nc.vector.tensor_copy(stg[:, c * 512:(c + 1) * 512], pst[:])
                nc.sync.dma_start(
                    out_d[ob * 128:(ob + 1) * 128, s0: s0 + S], stg[:],
                )


_NC_CACHE = None


def _get_nc():
    global _NC_CACHE
    if _NC_CACHE is None:
        _NC_CACHE = build_kernel()
    return _NC_CACHE


def kernel(**inputs) -> np.ndarray:
    x = np.ascontiguousarray(
        np.asarray(inputs["hidden_states"], np.float32).reshape(BS, D))
    Wq = np.asarray(inputs["Wq"], np.float32)
    Wk = np.asarray(inputs["Wk"], np.float32)
    Wv = np.asarray(inputs["Wv"], np.float32)
    Wo = np.asarray(inputs["Wo"], np.float32)
    bq = np.asarray(inputs["bq"], np.float32)
    bk = np.asarray(inputs["bk"], np.float32)
    bv = np.asarray(inputs["bv"], np.float32)
    bo = np.asarray(inputs["bo"], np.float32)

    nc = _get_nc()
    in_maps = []
    for c in range(N_CORES):
        js = slice(c * J, (c + 1) * J)
        in_maps.append({
            "x": x,
            "wq": np.ascontiguousarray(Wq[:, js]),
            "wk": np.ascontiguousarray(Wk[:, js]),
            "wv": np.ascontiguousarray(Wv[:, js]),
            "wo": np.ascontiguousarray(Wo[js, :]),
            "bq": np.ascontiguousarray(bq[js]),
            "bk": np.ascontiguousarray(bk[js]),
            "bv": np.ascontiguousarray(bv[js]),
        })

    res = run_bass_kernel_spmd(nc, in_maps, core_ids=list(range(N_CORES)))
    out_t = np.zeros((D, BS), np.float64)
    for c in range(N_CORES):
        out_t += res.results[c]["out_t"].astype(np.float64)
    out = out_t.T.astype(np.float32) + bo[None, :]
    return out.reshape(B, S, D)


if __name__ == "__main__":
    rng = np.random.default_rng(0)
    ins = {
        "hidden_states": rng.standard_normal((B, S, D), np.float32),
        "Wq": rng.standard_normal((D, D), np.float32) * 0.02,
        "bq": np.zeros(D, np.float32),
        "Wk": rng.standard_normal((D, D), np.float32) * 0.02,
        "bk": np.zeros(D, np.float32),
        "Wv": rng.standard_normal((D, D), np.float32) * 0.02,
        "bv": np.zeros(D, np.float32),
        "Wo": rng.standard_normal((D, D), np.float32) * 0.02,
        "bo": np.zeros(D, np.float32),
    }
    out = kernel(**ins)
    print("out", out.shape, out.dtype, float(np.abs(out).mean()))



# revision 7
# speedup vs baseline: 1.5987x; 1.5987x over previous
"""Trainium2 Bass kernel: GPT-2-style causal multi-head attention.

Problem: B=4, S=2048, D=1024, H=16 heads (head_dim 64), fp32 in/out.
  q/k/v = x @ W{q,k,v} + b{q,k,v}; causal softmax attention; out = attn_out @ Wo + bo.

Sharding (8 cores): tensor-parallel over heads - each core owns 2 heads
(128 feature dims). Wq/Wk/Wv column-sliced, Wo row-sliced per core. Each core
computes a partial o_proj output (transposed, [D, B*S] bf16); the host sums
the 8 partials in fp32, transposes, and adds bo.

v2 design (vs v1 baseline at 751us):
 - bf16 everywhere on device (fp32 PSUM accumulation). Host pre-transposes
   and pre-casts x to x^T bf16, so no on-chip x transposes at all.
 - scores for the 2 heads run as row-packed concurrent matmuls
   (head0 contraction rows 0-63 / head1 rows 64-127 via auto tile_position),
   doubling PE array utilization of the K=64 score matmuls.
 - V natural ([k, d] layout + ones column for the softmax denominator) is
   produced by XBAR DMA-transpose from v^T, not PE transposes.
 - single exp activation per k-block covering both heads' score tiles
   ([128, 1024] PSUM span) to amortize ACT instruction overhead.
 - software-pipelined emission: per 512-wide q-chunk "step", the next
   chunk's QKV projection matmuls and the previous chunk's o_proj matmuls
   are interleaved as fillers between score/AV matmuls so the PE never
   idles long enough for the HAM clock gate to re-throttle (3.4us).
"""

import sys

sys.path.insert(0, "/opt/trn_rl_repo")

import numpy as np

import concourse.bass as bass
import concourse.bacc as bacc
import concourse.tile as tile
import concourse.mybir as mybir
from concourse.bass_utils import run_bass_kernel_spmd

F32 = mybir.dt.float32
BF16 = mybir.dt.bfloat16

B, S, D, H = 4, 2048, 1024, 16
HD = D // H  # 64
N_CORES = 8
HPC = H // N_CORES  # heads per core = 2
J = HPC * HD  # per-core feature dims = 128
BS = B * S  # 8192
CH = 512  # q-chunk width
NCH = S // CH  # 4 chunks per batch
NU = B * NCH  # 16 chunk units total
NKB = S // 128  # k-blocks per batch


def build_kernel():
    nc = bacc.Bacc(
        "TRN2", target_bir_lowering=False, debug=False, enable_asserts=False,
        num_devices=N_CORES,
    )

    xT_d = nc.dram_tensor("xT", [D, BS], BF16, kind="ExternalInput").ap()
    wq_d = nc.dram_tensor("wq", [D, J], BF16, kind="ExternalInput").ap()
    wk_d = nc.dram_tensor("wk", [D, J], BF16, kind="ExternalInput").ap()
    wv_d = nc.dram_tensor("wv", [D, J], BF16, kind="ExternalInput").ap()
    wo_d = nc.dram_tensor("wo", [J, D], BF16, kind="ExternalInput").ap()
    bq_d = nc.dram_tensor("bq", [J], F32, kind="ExternalInput").ap()
    bk_d = nc.dram_tensor("bk", [J], F32, kind="ExternalInput").ap()
    bv_d = nc.dram_tensor("bv", [J], F32, kind="ExternalInput").ap()
    out_d = nc.dram_tensor("out_t", [D, BS], BF16, kind="ExternalOutput").ap()

    with tile.TileContext(nc) as tc:
        _emit(tc, nc, xT_d, wq_d, wk_d, wv_d, wo_d, bq_d, bk_d, bv_d, out_d)

    nc.compile()
    return nc


def _emit(tc, nc, xT_d, wq_d, wk_d, wv_d, wo_d, bq_d, bk_d, bv_d, out_d):
    from contextlib import ExitStack

    ADD = mybir.AluOpType.add

    ctx = ExitStack()
    with ctx:
        const = ctx.enter_context(tc.tile_pool(name="const", bufs=1))
        wpool = ctx.enter_context(tc.tile_pool(name="w", bufs=1))
        xtp = ctx.enter_context(tc.tile_pool(name="xtp", bufs=1))
        projp = ctx.enter_context(tc.tile_pool(name="projp", bufs=1))
        vep = ctx.enter_context(tc.tile_pool(name="vep", bufs=1))
        pp = ctx.enter_context(tc.tile_pool(name="pp", bufs=1))
        aotp = ctx.enter_context(tc.tile_pool(name="aotp", bufs=1))
        smallp = ctx.enter_context(tc.tile_pool(name="smallp", bufs=1))
        stgp = ctx.enter_context(tc.tile_pool(name="stgp", bufs=1))
        ps_sc = ctx.enter_context(tc.tile_pool(name="ps_sc", bufs=1, space="PSUM"))
        ps_av = ctx.enter_context(tc.tile_pool(name="ps_av", bufs=1, space="PSUM"))
        ps_sh = ctx.enter_context(tc.tile_pool(name="ps_sh", bufs=1, space="PSUM"))

        # --- constants ---------------------------------------------------
        # causal mask for diagonal 128x128 blocks of S^T[k, q]:
        # keep (1.0) where k <= q i.e. f - p >= 0
        mask_f = const.tile([128, 128], F32, tag="mask_f")
        nc.gpsimd.memset(mask_f[:], 1.0)
        nc.gpsimd.affine_select(
            mask_f[:], mask_f[:], pattern=[[1, 128]],
            compare_op=mybir.AluOpType.is_ge, fill=0.0,
            base=0, channel_multiplier=-1,
        )
        mask = const.tile([128, 128], BF16, tag="mask")
        nc.vector.tensor_copy(mask[:], mask_f[:])
        # ones [128, 16] bf16 for the ve ones-columns (softmax denominators)
        ones16 = const.tile([128, 16], BF16, tag="ones16")
        nc.gpsimd.memset(ones16[:], 1.0)
        # ones [128, 64] bf16; row 64 is the lhsT of the recip-broadcast mm
        onesM = const.tile([128, 64], BF16, tag="onesM")
        nc.gpsimd.memset(onesM[:], 1.0)
        # identity (bf16) for PE transposes of v^T -> V natural
        ident_f = const.tile([128, 128], F32, tag="ident_f")
        nc.gpsimd.memset(ident_f[:], 1.0)
        nc.gpsimd.affine_select(
            ident_f[:], ident_f[:], pattern=[[1, 128]],
            compare_op=mybir.AluOpType.is_equal, fill=0.0,
            base=0, channel_multiplier=-1,
        )
        ident = const.tile([128, 128], BF16, tag="ident")
        nc.vector.tensor_copy(ident[:], ident_f[:])

        # --- weights (already bf16 + pre-sliced on host) ----------------
        w_sb = {}
        for name, wd in (("q", wq_d), ("k", wk_d), ("v", wv_d)):
            t = wpool.tile([128, 8 * 128], BF16, tag=f"w{name}", name=f"w{name}")
            for ib in range(8):
                nc.sync.dma_start(
                    t[:, ib * 128:(ib + 1) * 128],
                    wd[ib * 128:(ib + 1) * 128, :])
            w_sb[name] = t
        wo_sb = wpool.tile([J, D], BF16, tag="wo")
        nc.sync.dma_start(wo_sb[:], wo_d[:, :])

        bias = {}
        for name, bd in (("q", bq_d), ("k", bk_d), ("v", bv_d)):
            t = const.tile([J, 1], F32, tag=f"b{name}", name=f"b{name}")
            nc.sync.dma_start(t[:], bd.rearrange("(p o) -> p o", o=1))
            bias[name] = t

        # --- pipeline state ---------------------------------------------
        xt_tiles = {}    # (u, ib) -> [128, 512] bf16 x^T chunk tiles
        proj_t = {}      # (name, b) -> [128, 2048] bf16 q^T/k^T/v^T
        ve_t = {}        # (b, h) -> [128, 16*65] bf16 V natural + ones cols
        aot_t = {}       # u -> [128, 512] bf16 normalized attn-out^T
        qkv_ps = {}      # name -> pending psum tile during split emission

        def emit_xt_dma(b):
            # whole-batch x^T tiles: fewer, larger DMAs (4KB/partition each)
            for ib in range(8):
                t = xtp.tile([128, S], BF16, tag="xt", name="xt", bufs=16)
                nc.sync.dma_start(
                    t[:], xT_d[ib * 128:(ib + 1) * 128, b * S:(b + 1) * S])
                xt_tiles[(b, ib)] = t

        def qkv_unit(u, name, half):
            b, c = divmod(u, NCH)
            if half == 0 and name == "q" and c == 0:
                # new batch: allocate proj + ve tiles
                for nm in ("q", "k", "v"):
                    proj_t[(nm, b)] = projp.tile(
                        [128, S], BF16, tag=f"p{nm}", name=f"p{nm}", bufs=2)
                for h in range(HPC):
                    ve = vep.tile([128, NKB * 65], BF16, tag=f"ve{h}",
                                  name=f"ve{h}", bufs=2)
                    nc.vector.tensor_copy(
                        ve[:].rearrange("p (nb c) -> p nb c", c=65)[:, :, 64:65],
                        ones16[:].rearrange("p (a o) -> p a o", o=1),
                    )
                    ve_t[(b, h)] = ve
            if half == 0:
                ps = ps_sh.tile([128, CH], F32, tag="sh", name="qkv_ps", bufs=2)
                qkv_ps[name] = ps
                for ib in range(4):
                    nc.tensor.matmul(
                        ps[:], w_sb[name][:, ib * 128:(ib + 1) * 128],
                        xt_tiles[(b, ib)][:, c * CH:(c + 1) * CH],
                        start=(ib == 0), stop=False,
                    )
            else:
                ps = qkv_ps[name]
                for ib in range(4, 8):
                    nc.tensor.matmul(
                        ps[:], w_sb[name][:, ib * 128:(ib + 1) * 128],
                        xt_tiles[(b, ib)][:, c * CH:(c + 1) * CH],
                        start=False, stop=(ib == 7),
                    )
                # evac PSUM -> SBUF bf16 with per-partition bias add
                nc.vector.tensor_scalar(
                    proj_t[(name, b)][:, c * CH:(c + 1) * CH], ps[:],
                    bias[name][:], None, ADD,
                )
                if name == "v":
                    # V natural via PE transpose: one [128,128] transpose per
                    # k-block yields both heads' V columns
                    pv = proj_t[("v", b)]
                    for kb in range(4 * c, 4 * c + 4):
                        pst = ps_sh.tile([128, 256], BF16, tag="sh",
                                         name="vtp", bufs=2)
                        nc.tensor.transpose(
                            pst[:, 0:128],
                            pv[:, kb * 128:(kb + 1) * 128], ident[:],
                        )
                        for h in range(HPC):
                            nc.vector.tensor_copy(
                                ve_t[(b, h)][:, kb * 65: kb * 65 + 64],
                                pst[:, h * 64:(h + 1) * 64],
                            )

        def oproj_unit(u, ob):
            b, c = divmod(u, NCH)
            ps = ps_sh.tile([128, CH], F32, tag="sh", name="op_ps", bufs=2)
            nc.tensor.matmul(
                ps[:], wo_sb[:, ob * 128:(ob + 1) * 128], aot_t[u][:],
                start=True, stop=True,
            )
            stg = stgp.tile([128, CH], BF16, tag="stg", name="stg", bufs=4)
            nc.vector.tensor_copy(stg[:], ps[:])
            nc.sync.dma_start(
                out_d[ob * 128:(ob + 1) * 128,
                      b * S + c * CH: b * S + (c + 1) * CH],
                stg[:],
            )

        def emit_attention(u, fillers):
            b, c = divmod(u, NCH)
            nkb = 4 * c + 4
            qt = proj_t[("q", b)]
            kt = proj_t[("k", b)]
            acc = [
                ps_av.tile([128, CH], F32, tag="av", name="acc0", bufs=2),
                ps_av.tile([128, CH], F32, tag="av", name="acc1", bufs=2),
            ]
            fill_i = 0

            def run_fillers(n):
                nonlocal fill_i
                for _ in range(n):
                    if fill_i < len(fillers):
                        fillers[fill_i]()
                        fill_i += 1

            prev = None  # (p tile, kb, lo)
            for kb in range(nkb):
                lo = max(0, 128 * kb - CH * c)
                st = ps_sc.tile([128, 2 * CH], F32, tag="sc", name="st", bufs=2)
                for h in range(HPC):
                    nc.tensor.matmul(
                        st[:, h * CH + lo:(h + 1) * CH],
                        kt[h * 64:(h + 1) * 64, kb * 128:(kb + 1) * 128],
                        qt[h * 64:(h + 1) * 64, c * CH + lo:(c + 1) * CH],
                        start=True, stop=True,
                    )
                p = pp.tile([128, 2 * CH], BF16, tag="p", name="p", bufs=5)
                nc.scalar.activation(
                    p[:, lo:2 * CH], st[:, lo:2 * CH],
                    mybir.ActivationFunctionType.Exp, scale=0.125,
                )
                if kb >= 4 * c:  # diagonal block: mask lower triangle
                    for h in range(HPC):
                        nc.vector.tensor_mul(
                            p[:, h * CH + lo: h * CH + lo + 128],
                            p[:, h * CH + lo: h * CH + lo + 128],
                            mask[:],
                        )
                if prev is not None:
                    pprev, kbp, lop = prev
                    for h in range(HPC):
                        nc.tensor.matmul(
                            acc[h][0:65, lop:CH],
                            ve_t[(b, h)][:, kbp * 65: kbp * 65 + 65],
                            pprev[:, h * CH + lop:(h + 1) * CH],
                            start=(kbp == 0), stop=(kbp == nkb - 1),
                        )
                run_fillers(
                    (len(fillers) - fill_i + (nkb - 1 - kb)) // max(1, nkb - kb))
                prev = (p, kb, lo)
            pprev, kbp, lop = prev
            for h in range(HPC):
                nc.tensor.matmul(
                    acc[h][0:65, lop:CH],
                    ve_t[(b, h)][:, kbp * 65: kbp * 65 + 65],
                    pprev[:, h * CH + lop:(h + 1) * CH],
                    start=(kbp == 0), stop=(kbp == nkb - 1),
                )
            # normalize: row 64 of acc = softmax denominators
            aot = aotp.tile([128, CH], BF16, tag="aot", name="aot", bufs=3)
            aot_t[u] = aot
            for h in range(HPC):
                rec = smallp.tile([65, CH], BF16, tag="rec", name="rec", bufs=2)
                with nc.allow_low_precision(reason="softmax recip in bf16"):
                    nc.vector.reciprocal(rec[64:65, :], acc[h][64:65, :])
                bcp = ps_sh.tile([128, CH], F32, tag="sh", name="bcp", bufs=2)
                nc.tensor.matmul(
                    bcp[0:64, :], onesM[64:65, :], rec[64:65, :],
                    start=True, stop=True,
                )
                bct = smallp.tile([64, CH], BF16, tag="bct", name="bct", bufs=2)
                nc.vector.tensor_copy(bct[:], bcp[0:64, :])
                if h == 0:
                    nc.vector.tensor_mul(aot[0:64, :], acc[h][0:64, :], bct[:])
                else:
                    tmp = smallp.tile([64, CH], BF16, tag="tmp", name="tmp",
                                      bufs=2)
                    nc.vector.tensor_mul(tmp[:], acc[h][0:64, :], bct[:])
                    # partition shift 0-63 -> 64-127 via SBUF->SBUF DMA
                    nc.sync.dma_start(aot[64:128, :], tmp[:])
            run_fillers(len(fillers))

        # --- steps --------------------------------------------------------
        emit_xt_dma(0)
        for s in range(NU + 2):
            if s % NCH == 0 and s // NCH + 1 < B:
                emit_xt_dma(s // NCH + 1)
            fillers = []
            if s < NU:
                for name in ("q", "k", "v"):
                    for half in range(2):
                        fillers.append(
                            lambda u=s, n=name, hf=half: qkv_unit(u, n, hf))
            if 2 <= s:
                for ob in range(8):
                    fillers.append(lambda u=s - 2, o=ob: oproj_unit(u, o))
            if 1 <= s <= NU:
                emit_attention(s - 1, fillers)
            else:
                for f in fillers:
                    f()


_NC_CACHE = None


def _get_nc():
    global _NC_CACHE
    if _NC_CACHE is None:
        _NC_CACHE = build_kernel()
    return _NC_CACHE


def _to_bf16(a):
    import ml_dtypes
    return np.asarray(a).astype(ml_dtypes.bfloat16)


def make_in_maps(inputs):
    x = np.asarray(inputs["hidden_states"], np.float32).reshape(BS, D)
    xT = np.ascontiguousarray(_to_bf16(x).T)  # [D, BS] bf16
    Wq = _to_bf16(inputs["Wq"])
    Wk = _to_bf16(inputs["Wk"])
    Wv = _to_bf16(inputs["Wv"])
    Wo = _to_bf16(inputs["Wo"])
    bq = np.asarray(inputs["bq"], np.float32)
    bk = np.asarray(inputs["bk"], np.float32)
    bv = np.asarray(inputs["bv"], np.float32)

    in_maps = []
    for c in range(N_CORES):
        js = slice(c * J, (c + 1) * J)
        in_maps.append({
            "xT": xT,
            "wq": np.ascontiguousarray(Wq[:, js]),
            "wk": np.ascontiguousarray(Wk[:, js]),
            "wv": np.ascontiguousarray(Wv[:, js]),
            "wo": np.ascontiguousarray(Wo[js, :]),
            "bq": np.ascontiguousarray(bq[js]),
            "bk": np.ascontiguousarray(bk[js]),
            "bv": np.ascontiguousarray(bv[js]),
        })
    return in_maps


def gather_out(results, bo):
    out_t = np.zeros((D, BS), np.float32)
    for c in range(N_CORES):
        out_t += results[c]["out_t"].astype(np.float32)
    out = out_t.T + np.asarray(bo, np.float32)[None, :]
    return out.reshape(B, S, D).astype(np.float32)


def kernel(**inputs) -> np.ndarray:
    nc = _get_nc()
    in_maps = make_in_maps(inputs)
    res = run_bass_kernel_spmd(nc, in_maps, core_ids=list(range(N_CORES)))
    return gather_out(res.results, inputs["bo"])


if __name__ == "__main__":
    rng = np.random.default_rng(0)
    ins = {
        "hidden_states": rng.standard_normal((B, S, D), np.float32),
        "Wq": rng.standard_normal((D, D), np.float32) * 0.02,
        "bq": np.zeros(D, np.float32),
        "Wk": rng.standard_normal((D, D), np.float32) * 0.02,
        "bk": np.zeros(D, np.float32),
        "Wv": rng.standard_normal((D, D), np.float32) * 0.02,
        "bv": np.zeros(D, np.float32),
        "Wo": rng.standard_normal((D, D), np.float32) * 0.02,
        "bo": np.zeros(D, np.float32),
    }
    out = kernel(**ins)
    print("out", out.shape, out.dtype, float(np.abs(out).mean()))


# revision 15
# speedup vs baseline: 1.9148x; 1.1977x over previous
"""Trainium2 Bass kernel: GPT-2-style causal multi-head attention.

Problem: B=4, S=2048, D=1024, H=16 heads (head_dim 64), fp32 in/out.
  q/k/v = x @ W{q,k,v} + b{q,k,v}; causal softmax attention; out = attn_out @ Wo + bo.

Sharding (8 cores): tensor-parallel over heads - each core owns 2 heads
(128 feature dims). Wq/Wk/Wv column-sliced, Wo row-sliced per core. Each core
computes a partial o_proj output (transposed, [D, B*S] bf16); the host sums
the 8 partials in fp32, transposes, and adds bo.

v2 design (vs v1 baseline at 751us):
 - bf16 everywhere on device (fp32 PSUM accumulation). Host pre-transposes
   and pre-casts x to x^T bf16, so no on-chip x transposes at all.
 - scores for the 2 heads run as row-packed concurrent matmuls
   (head0 contraction rows 0-63 / head1 rows 64-127 via auto tile_position),
   doubling PE array utilization of the K=64 score matmuls.
 - V natural ([k, d] layout + ones column for the softmax denominator) is
   produced by XBAR DMA-transpose from v^T, not PE transposes.
 - single exp activation per k-block covering both heads' score tiles
   ([128, 1024] PSUM span) to amortize ACT instruction overhead.
 - software-pipelined emission: per 512-wide q-chunk "step", the next
   chunk's QKV projection matmuls and the previous chunk's o_proj matmuls
   are interleaved as fillers between score/AV matmuls so the PE never
   idles long enough for the HAM clock gate to re-throttle (3.4us).
"""

import sys

sys.path.insert(0, "/opt/trn_rl_repo")

import numpy as np

import concourse.bass as bass
import concourse.bacc as bacc
import concourse.tile as tile
import concourse.mybir as mybir
from concourse.bass_utils import run_bass_kernel_spmd

F32 = mybir.dt.float32
F32R = mybir.dt.float32r
BF16 = mybir.dt.bfloat16

FAST_RECIP = True  # reciprocal_approx_fast (1 pass) vs exact InstReciprocal

B, S, D, H = 4, 2048, 1024, 16
HD = D // H  # 64
N_CORES = 8
HPC = H // N_CORES  # heads per core = 2
J = HPC * HD  # per-core feature dims = 128
BS = B * S  # 8192
CH = 512  # q-chunk width
NCH = S // CH  # 4 chunks per batch
NU = B * NCH  # 16 chunk units total
NKB = S // 128  # k-blocks per batch


def build_kernel():
    nc = bacc.Bacc(
        "TRN2", target_bir_lowering=False, debug=False, enable_asserts=False,
        num_devices=N_CORES,
    )

    xT_d = nc.dram_tensor("xT", [D, BS], BF16, kind="ExternalInput").ap()
    wq_d = nc.dram_tensor("wq", [D, J], BF16, kind="ExternalInput").ap()
    wk_d = nc.dram_tensor("wk", [D, J], BF16, kind="ExternalInput").ap()
    wv_d = nc.dram_tensor("wv", [D, J], BF16, kind="ExternalInput").ap()
    wo_d = nc.dram_tensor("wo", [J, D], BF16, kind="ExternalInput").ap()
    bq_d = nc.dram_tensor("bq", [J], F32, kind="ExternalInput").ap()
    bk_d = nc.dram_tensor("bk", [J], F32, kind="ExternalInput").ap()
    bv_d = nc.dram_tensor("bv", [J], F32, kind="ExternalInput").ap()
    out_d = nc.dram_tensor("out_t", [D, BS], BF16, kind="ExternalOutput").ap()

    with tile.TileContext(nc) as tc:
        _emit(tc, nc, xT_d, wq_d, wk_d, wv_d, wo_d, bq_d, bk_d, bv_d, out_d)

    nc.compile()
    return nc


def _emit(tc, nc, xT_d, wq_d, wk_d, wv_d, wo_d, bq_d, bk_d, bv_d, out_d):
    from contextlib import ExitStack

    ADD = mybir.AluOpType.add

    ctx = ExitStack()
    with ctx:
        const = ctx.enter_context(tc.tile_pool(name="const", bufs=1))
        wpool = ctx.enter_context(tc.tile_pool(name="w", bufs=1))
        xtp = ctx.enter_context(tc.tile_pool(name="xtp", bufs=1))
        projp = ctx.enter_context(tc.tile_pool(name="projp", bufs=1))
        vep = ctx.enter_context(tc.tile_pool(name="vep", bufs=1))
        pp = ctx.enter_context(tc.tile_pool(name="pp", bufs=1))
        aotp = ctx.enter_context(tc.tile_pool(name="aotp", bufs=1))
        smallp = ctx.enter_context(tc.tile_pool(name="smallp", bufs=1))
        stgp = ctx.enter_context(tc.tile_pool(name="stgp", bufs=1))
        ps_sc = ctx.enter_context(tc.tile_pool(name="ps_sc", bufs=1, space="PSUM"))
        ps_av = ctx.enter_context(tc.tile_pool(name="ps_av", bufs=1, space="PSUM"))
        ps_sh = ctx.enter_context(tc.tile_pool(name="ps_sh", bufs=1, space="PSUM"))

        # --- constants ---------------------------------------------------
        # causal mask for diagonal 128x128 blocks of S^T[k, q]:
        # keep (1.0) where k <= q i.e. f - p >= 0
        mask_f = const.tile([128, 128], F32, tag="mask_f")
        nc.gpsimd.memset(mask_f[:], 1.0)
        nc.gpsimd.affine_select(
            mask_f[:], mask_f[:], pattern=[[1, 128]],
            compare_op=mybir.AluOpType.is_ge, fill=0.0,
            base=0, channel_multiplier=-1,
        )
        mask = const.tile([128, 128], BF16, tag="mask")
        nc.vector.tensor_copy(mask[:], mask_f[:])
        # ones [128, 16] bf16 for the ve ones-columns (softmax denominators)
        ones16 = const.tile([128, 16], BF16, tag="ones16")
        nc.gpsimd.memset(ones16[:], 1.0)
        # ones [128, 64]; row 64 is the lhsT of the recip-broadcast mm
        # (f32r so it can pair with the f32r-rounded denominator row as rhs;
        #  memset can't target f32r, so round via DVE copy)
        onesMf = const.tile([128, 64], F32, tag="onesMf")
        nc.gpsimd.memset(onesMf[:], 1.0)
        onesM = const.tile([128, 64], F32R, tag="onesM")
        nc.vector.tensor_copy(onesM[:], onesMf[:])
        # identity (bf16) for PE transposes of v^T -> V natural
        ident_f = const.tile([128, 128], F32, tag="ident_f")
        nc.gpsimd.memset(ident_f[:], 1.0)
        nc.gpsimd.affine_select(
            ident_f[:], ident_f[:], pattern=[[1, 128]],
            compare_op=mybir.AluOpType.is_equal, fill=0.0,
            base=0, channel_multiplier=-1,
        )
        ident = const.tile([128, 128], BF16, tag="ident")
        nc.vector.tensor_copy(ident[:], ident_f[:])

        # --- weights (already bf16 + pre-sliced on host) ----------------
        w_sb = {}
        for name, wd in (("q", wq_d), ("k", wk_d), ("v", wv_d)):
            t = wpool.tile([128, 8 * 128], BF16, tag=f"w{name}", name=f"w{name}")
            for ib in range(8):
                nc.sync.dma_start(
                    t[:, ib * 128:(ib + 1) * 128],
                    wd[ib * 128:(ib + 1) * 128, :])
            w_sb[name] = t
        wo_sb = wpool.tile([J, D], BF16, tag="wo")
        nc.sync.dma_start(wo_sb[:], wo_d[:, :])

        bias = {}
        for name, bd in (("q", bq_d), ("k", bk_d), ("v", bv_d)):
            t = const.tile([J, 1], F32, tag=f"b{name}", name=f"b{name}")
            nc.sync.dma_start(t[:], bd.rearrange("(p o) -> p o", o=1))
            bias[name] = t

        # --- pipeline state ---------------------------------------------
        xt_tiles = {}    # (u, ib) -> [128, 512] bf16 x^T chunk tiles
        proj_t = {}      # (name, b) -> [128, 2048] bf16 q^T/k^T/v^T
        ve_t = {}        # (b, h) -> [128, 16*65] bf16 V natural + ones cols
        aot_t = {}       # u -> [128, 512] bf16 normalized attn-out^T
        qkv_ps = {}      # name -> pending psum tile during split emission

        def emit_xt_dma(b):
            # whole-batch x^T tiles: fewer, larger DMAs (4KB/partition each)
            for ib in range(8):
                t = xtp.tile([128, S], BF16, tag="xt", name="xt", bufs=16)
                nc.sync.dma_start(
                    t[:], xT_d[ib * 128:(ib + 1) * 128, b * S:(b + 1) * S])
                xt_tiles[(b, ib)] = t

        def qkv_unit(u, name, half):
            b, c = divmod(u, NCH)
            if half == 0 and name == "q" and c == 0:
                # new batch: allocate proj + ve tiles
                for nm in ("q", "k", "v"):
                    proj_t[(nm, b)] = projp.tile(
                        [128, S], BF16, tag=f"p{nm}", name=f"p{nm}", bufs=2)
                for h in range(HPC):
                    ve = vep.tile([128, NKB * 65], BF16, tag=f"ve{h}",
                                  name=f"ve{h}", bufs=2)
                    nc.vector.tensor_copy(
                        ve[:].rearrange("p (nb c) -> p nb c", c=65)[:, :, 64:65],
                        ones16[:].rearrange("p (a o) -> p a o", o=1),
                    )
                    ve_t[(b, h)] = ve
            if half == 0:
                ps = ps_sh.tile([128, CH], F32, tag="sh", name="qkv_ps", bufs=2)
                qkv_ps[name] = ps
                for ib in range(4):
                    nc.tensor.matmul(
                        ps[:], w_sb[name][:, ib * 128:(ib + 1) * 128],
                        xt_tiles[(b, ib)][:, c * CH:(c + 1) * CH],
                        start=(ib == 0), stop=False,
                    )
            else:
                ps = qkv_ps[name]
                for ib in range(4, 8):
                    nc.tensor.matmul(
                        ps[:], w_sb[name][:, ib * 128:(ib + 1) * 128],
                        xt_tiles[(b, ib)][:, c * CH:(c + 1) * CH],
                        start=False, stop=(ib == 7),
                    )
                # evac PSUM -> SBUF bf16 with per-partition bias add
                nc.vector.tensor_scalar(
                    proj_t[(name, b)][:, c * CH:(c + 1) * CH], ps[:],
                    bias[name][:], None, ADD,
                )
                if name == "v":
                    # V natural via PE transpose: one [128,128] transpose per
                    # k-block yields both heads' V columns
                    pv = proj_t[("v", b)]
                    for kb in range(4 * c, 4 * c + 4):
                        pst = ps_sh.tile([128, 256], BF16, tag="sh",
                                         name="vtp", bufs=2)
                        nc.tensor.transpose(
                            pst[:, 0:128],
                            pv[:, kb * 128:(kb + 1) * 128], ident[:],
                        )
                        for h in range(HPC):
                            nc.vector.tensor_copy(
                                ve_t[(b, h)][:, kb * 65: kb * 65 + 64],
                                pst[:, h * 64:(h + 1) * 64],
                            )

        def oproj_unit(u, ob):
            b, c = divmod(u, NCH)
            ps = ps_sh.tile([128, CH], F32, tag="sh", name="op_ps", bufs=2)
            nc.tensor.matmul(
                ps[:], wo_sb[:, ob * 128:(ob + 1) * 128], aot_t[u][:],
                start=True, stop=True,
            )
            stg = stgp.tile([128, CH], BF16, tag="stg", name="stg", bufs=4)
            nc.vector.tensor_copy(stg[:], ps[:])
            nc.sync.dma_start(
                out_d[ob * 128:(ob + 1) * 128,
                      b * S + c * CH: b * S + (c + 1) * CH],
                stg[:],
            )

        def emit_attention(u, fillers):
            b, c = divmod(u, NCH)
            nkb = 4 * c + 4
            qt = proj_t[("q", b)]
            kt = proj_t[("k", b)]
            acc = [
                ps_av.tile([128, CH], F32, tag="av", name="acc0", bufs=2),
                ps_av.tile([128, CH], F32, tag="av", name="acc1", bufs=2),
            ]
            fill_i = 0

            def run_fillers(n):
                nonlocal fill_i
                for _ in range(n):
                    if fill_i < len(fillers):
                        fillers[fill_i]()
                        fill_i += 1

            prev = None  # (p tile, kb, lo)
            for kb in range(nkb):
                lo = max(0, 128 * kb - CH * c)
                st = ps_sc.tile([128, 2 * CH], F32, tag="sc", name="st", bufs=2)
                for h in range(HPC):
                    nc.tensor.matmul(
                        st[:, h * CH + lo:(h + 1) * CH],
                        kt[h * 64:(h + 1) * 64, kb * 128:(kb + 1) * 128],
                        qt[h * 64:(h + 1) * 64, c * CH + lo:(c + 1) * CH],
                        start=True, stop=True,
                    )
                p = pp.tile([128, 2 * CH], BF16, tag="p", name="p", bufs=5)
                nc.scalar.activation(
                    p[:, lo:2 * CH], st[:, lo:2 * CH],
                    mybir.ActivationFunctionType.Exp, scale=0.125,
                )
                if kb >= 4 * c:  # diagonal block: mask lower triangle
                    for h in range(HPC):
                        nc.vector.tensor_mul(
                            p[:, h * CH + lo: h * CH + lo + 128],
                            p[:, h * CH + lo: h * CH + lo + 128],
                            mask[:],
                        )
                if prev is not None:
                    pprev, kbp, lop = prev
                    for h in range(HPC):
                        nc.tensor.matmul(
                            acc[h][0:65, lop:CH],
                            ve_t[(b, h)][:, kbp * 65: kbp * 65 + 65],
                            pprev[:, h * CH + lop:(h + 1) * CH],
                            start=(kbp == 0), stop=(kbp == nkb - 1),
                        )
                # pace fillers, holding a few back to cover the chunk tail
                # (last exp->AV waits and the normalize recip chain)
                run_fillers((len(fillers) - fill_i) // (nkb - kb + 3))
                prev = (p, kb, lo)
            pprev, kbp, lop = prev
            for h in range(HPC):
                nc.tensor.matmul(
                    acc[h][0:65, lop:CH],
                    ve_t[(b, h)][:, kbp * 65: kbp * 65 + 65],
                    pprev[:, h * CH + lop:(h + 1) * CH],
                    start=(kbp == 0), stop=(kbp == nkb - 1),
                )
            # normalize: row 64 of acc = softmax denominators
            aot = aotp.tile([128, CH], BF16, tag="aot", name="aot", bufs=3)
            aot_t[u] = aot
            for h in range(HPC):
                if FAST_RECIP:
                    # broadcast raw denominators d via the ones-matmul, then
                    # 1-pass NR approx recip on the full [64, CH] block (the
                    # single-row approx variant miscomputes on HW)
                    rec = smallp.tile([65, CH], F32R, tag="rec", name="rec",
                                      bufs=4)
                    nc.vector.tensor_copy(rec[64:65, :], acc[h][64:65, :])
                    bcp = ps_sh.tile([128, CH], F32, tag="sh", name="bcp",
                                     bufs=2)
                    nc.tensor.matmul(
                        bcp[0:64, :], onesM[64:65, :], rec[64:65, :],
                        start=True, stop=True,
                    )
                    bct = smallp.tile([64, CH], F32, tag="bct", name="bct",
                                      bufs=2)
                    nc.vector.tensor_copy(bct[:], bcp[0:64, :])
                    rr = smallp.tile([64, CH], F32, tag="rr", name="rr",
                                     bufs=2)
                    nc.vector.reciprocal_approx_fast(out=rr[:], in_=bct[:])
                else:
                    rec = smallp.tile([65, CH], F32R, tag="rec", name="rec",
                                      bufs=4)
                    with nc.allow_low_precision(reason="recip rounded to f32r"):
                        nc.vector.reciprocal(rec[64:65, :], acc[h][64:65, :])
                    bcp = ps_sh.tile([128, CH], F32, tag="sh", name="bcp",
                                     bufs=2)
                    nc.tensor.matmul(
                        bcp[0:64, :], onesM[64:65, :], rec[64:65, :],
                        start=True, stop=True,
                    )
                    rr = smallp.tile([64, CH], F32, tag="rr", name="rr",
                                     bufs=2)
                    nc.vector.tensor_copy(rr[:], bcp[0:64, :])
                if h == 0:
                    nc.vector.tensor_mul(aot[0:64, :], acc[h][0:64, :], rr[:])
                else:
                    tmp = smallp.tile([64, CH], BF16, tag="tmp", name="tmp",
                                      bufs=2)
                    nc.vector.tensor_mul(tmp[:], acc[h][0:64, :], rr[:])
                    # partition shift 0-63 -> 64-127 via SBUF->SBUF DMA
                    nc.sync.dma_start(aot[64:128, :], tmp[:])
                run_fillers(2)
            run_fillers(len(fillers))

        # --- steps --------------------------------------------------------
        emit_xt_dma(0)
        for s in range(NU + 2):
            if s % NCH == 0 and s // NCH + 1 < B:
                emit_xt_dma(s // NCH + 1)
            fillers = []
            if s < NU:
                for name in ("q", "k", "v"):
                    for half in range(2):
                        fillers.append(
                            lambda u=s, n=name, hf=half: qkv_unit(u, n, hf))
            if 2 <= s:
                for ob in range(8):
                    fillers.append(lambda u=s - 2, o=ob: oproj_unit(u, o))
            if 1 <= s <= NU:
                emit_attention(s - 1, fillers)
            else:
                for f in fillers:
                    f()


_NC_CACHE = None


def _get_nc():
    global _NC_CACHE
    if _NC_CACHE is None:
        _NC_CACHE = build_kernel()
    return _NC_CACHE


def _to_bf16(a):
    import ml_dtypes
    return np.asarray(a).astype(ml_dtypes.bfloat16)


def make_in_maps(inputs):
    x = np.asarray(inputs["hidden_states"], np.float32).reshape(BS, D)
    xT = np.ascontiguousarray(_to_bf16(x).T)  # [D, BS] bf16
    Wq = _to_bf16(inputs["Wq"])
    Wk = _to_bf16(inputs["Wk"])
    Wv = _to_bf16(inputs["Wv"])
    Wo = _to_bf16(inputs["Wo"])
    bq = np.asarray(inputs["bq"], np.float32)
    bk = np.asarray(inputs["bk"], np.float32)
    bv = np.asarray(inputs["bv"], np.float32)

    in_maps = []
    for c in range(N_CORES):
        js = slice(c * J, (c + 1) * J)
        in_maps.append({
            "xT": xT,
            "wq": np.ascontiguousarray(Wq[:, js]),
            "wk": np.ascontiguousarray(Wk[:, js]),
            "wv": np.ascontiguousarray(Wv[:, js]),
            "wo": np.ascontiguousarray(Wo[js, :]),
            "bq": np.ascontiguousarray(bq[js]),
            "bk": np.ascontiguousarray(bk[js]),
            "bv": np.ascontiguousarray(bv[js]),
        })
    return in_maps


def gather_out(results, bo):
    out_t = np.zeros((D, BS), np.float32)
    for c in range(N_CORES):
        out_t += results[c]["out_t"].astype(np.float32)
    out = out_t.T + np.asarray(bo, np.float32)[None, :]
    return out.reshape(B, S, D).astype(np.float32)


def kernel(**inputs) -> np.ndarray:
    nc = _get_nc()
    in_maps = make_in_maps(inputs)
    res = run_bass_kernel_spmd(nc, in_maps, core_ids=list(range(N_CORES)))
    return gather_out(res.results, inputs["bo"])


if __name__ == "__main__":
    rng = np.random.default_rng(0)
    ins = {
        "hidden_states": rng.standard_normal((B, S, D), np.float32),
        "Wq": rng.standard_normal((D, D), np.float32) * 0.02,
        "bq": np.zeros(D, np.float32),
        "Wk": rng.standard_normal((D, D), np.float32) * 0.02,
        "bk": np.zeros(D, np.float32),
        "Wv": rng.standard_normal((D, D), np.float32) * 0.02,
        "bv": np.zeros(D, np.float32),
        "Wo": rng.standard_normal((D, D), np.float32) * 0.02,
        "bo": np.zeros(D, np.float32),
    }
    out = kernel(**ins)
    print("out", out.shape, out.dtype, float(np.abs(out).mean()))


# revision 21
# speedup vs baseline: 2.0516x; 1.0715x over previous
"""Trainium2 Bass kernel: GPT-2-style causal multi-head attention.

Problem: B=4, S=2048, D=1024, H=16 heads (head_dim 64), fp32 in/out.
  q/k/v = x @ W{q,k,v} + b{q,k,v}; causal softmax attention; out = attn_out @ Wo + bo.

Sharding (8 cores): tensor-parallel over heads - each core owns 2 heads
(128 feature dims). Wq/Wk/Wv column-sliced, Wo row-sliced per core. Each core
computes a partial o_proj output (transposed, [D, B*S] bf16); the host sums
the 8 partials in fp32, transposes, and adds bo.

v2 design (vs v1 baseline at 751us):
 - bf16 everywhere on device (fp32 PSUM accumulation). Host pre-transposes
   and pre-casts x to x^T bf16, so no on-chip x transposes at all.
 - scores for the 2 heads run as row-packed concurrent matmuls
   (head0 contraction rows 0-63 / head1 rows 64-127 via auto tile_position),
   doubling PE array utilization of the K=64 score matmuls.
 - V natural ([k, d] layout + ones column for the softmax denominator) is
   produced by XBAR DMA-transpose from v^T, not PE transposes.
 - single exp activation per k-block covering both heads' score tiles
   ([128, 1024] PSUM span) to amortize ACT instruction overhead.
 - software-pipelined emission: per 512-wide q-chunk "step", the next
   chunk's QKV projection matmuls and the previous chunk's o_proj matmuls
   are interleaved as fillers between score/AV matmuls so the PE never
   idles long enough for the HAM clock gate to re-throttle (3.4us).
"""

import sys

sys.path.insert(0, "/opt/trn_rl_repo")

import numpy as np

import concourse.bass as bass
import concourse.bacc as bacc
import concourse.tile as tile
import concourse.mybir as mybir
from concourse.bass_utils import run_bass_kernel_spmd

F32 = mybir.dt.float32
F32R = mybir.dt.float32r
BF16 = mybir.dt.bfloat16

FAST_RECIP = True  # reciprocal_approx_fast (1 pass) vs exact InstReciprocal

B, S, D, H = 4, 2048, 1024, 16
HD = D // H  # 64
N_CORES = 8
HPC = H // N_CORES  # heads per core = 2
J = HPC * HD  # per-core feature dims = 128
BS = B * S  # 8192
CH = 512  # q-chunk width
NCH = S // CH  # 4 chunks per batch
NU = B * NCH  # 16 chunk units total
NKB = S // 128  # k-blocks per batch


def build_kernel():
    nc = bacc.Bacc(
        "TRN2", target_bir_lowering=False, debug=False, enable_asserts=False,
        num_devices=N_CORES,
    )

    xT_d = nc.dram_tensor("xT", [D, BS], BF16, kind="ExternalInput").ap()
    wq_d = nc.dram_tensor("wq", [D, J], BF16, kind="ExternalInput").ap()
    wk_d = nc.dram_tensor("wk", [D, J], BF16, kind="ExternalInput").ap()
    wv_d = nc.dram_tensor("wv", [D, J], BF16, kind="ExternalInput").ap()
    wo_d = nc.dram_tensor("wo", [J, D], BF16, kind="ExternalInput").ap()
    bq_d = nc.dram_tensor("bq", [J], F32, kind="ExternalInput").ap()
    bk_d = nc.dram_tensor("bk", [J], F32, kind="ExternalInput").ap()
    bv_d = nc.dram_tensor("bv", [J], F32, kind="ExternalInput").ap()
    out_d = nc.dram_tensor("out_t", [D, BS], BF16, kind="ExternalOutput").ap()

    with tile.TileContext(nc) as tc:
        _emit(tc, nc, xT_d, wq_d, wk_d, wv_d, wo_d, bq_d, bk_d, bv_d, out_d)

    nc.compile()
    return nc


def _emit(tc, nc, xT_d, wq_d, wk_d, wv_d, wo_d, bq_d, bk_d, bv_d, out_d):
    from contextlib import ExitStack

    ADD = mybir.AluOpType.add

    ctx = ExitStack()
    with ctx:
        const = ctx.enter_context(tc.tile_pool(name="const", bufs=1))
        wpool = ctx.enter_context(tc.tile_pool(name="w", bufs=1))
        xtp = ctx.enter_context(tc.tile_pool(name="xtp", bufs=1))
        projp = ctx.enter_context(tc.tile_pool(name="projp", bufs=1))
        vep = ctx.enter_context(tc.tile_pool(name="vep", bufs=1))
        pp = ctx.enter_context(tc.tile_pool(name="pp", bufs=1))
        aotp = ctx.enter_context(tc.tile_pool(name="aotp", bufs=1))
        smallp = ctx.enter_context(tc.tile_pool(name="smallp", bufs=1))
        stgp = ctx.enter_context(tc.tile_pool(name="stgp", bufs=1))
        ps_sc = ctx.enter_context(tc.tile_pool(name="ps_sc", bufs=1, space="PSUM"))
        ps_av = ctx.enter_context(tc.tile_pool(name="ps_av", bufs=1, space="PSUM"))
        ps_sh = ctx.enter_context(tc.tile_pool(name="ps_sh", bufs=1, space="PSUM"))

        # --- constants ---------------------------------------------------
        # causal mask for diagonal 128x128 blocks of S^T[k, q]:
        # keep (1.0) where k <= q i.e. f - p >= 0
        mask_f = const.tile([128, 128], F32, tag="mask_f")
        nc.gpsimd.memset(mask_f[:], 1.0)
        nc.gpsimd.affine_select(
            mask_f[:], mask_f[:], pattern=[[1, 128]],
            compare_op=mybir.AluOpType.is_ge, fill=0.0,
            base=0, channel_multiplier=-1,
        )
        mask = const.tile([128, 128], BF16, tag="mask")
        nc.vector.tensor_copy(mask[:], mask_f[:])
        # ones [128, 16] bf16 for the ve ones-columns (softmax denominators)
        ones16 = const.tile([128, 16], BF16, tag="ones16")
        nc.gpsimd.memset(ones16[:], 1.0)
        # ones [128, 64]; row 64 is the lhsT of the recip-broadcast mm
        # (f32r so it can pair with the f32r-rounded denominator row as rhs;
        #  memset can't target f32r, so round via DVE copy)
        onesMf = const.tile([128, 64], F32, tag="onesMf")
        nc.gpsimd.memset(onesMf[:], 1.0)
        onesM = const.tile([128, 64], F32R, tag="onesM")
        nc.vector.tensor_copy(onesM[:], onesMf[:])
        # identity (bf16) for PE transposes of v^T -> V natural
        ident_f = const.tile([128, 128], F32, tag="ident_f")
        nc.gpsimd.memset(ident_f[:], 1.0)
        nc.gpsimd.affine_select(
            ident_f[:], ident_f[:], pattern=[[1, 128]],
            compare_op=mybir.AluOpType.is_equal, fill=0.0,
            base=0, channel_multiplier=-1,
        )
        ident = const.tile([128, 128], BF16, tag="ident")
        nc.vector.tensor_copy(ident[:], ident_f[:])

        # --- weights (already bf16 + pre-sliced on host) ----------------
        w_sb = {}
        for name, wd in (("q", wq_d), ("k", wk_d), ("v", wv_d)):
            t = wpool.tile([128, 8 * 128], BF16, tag=f"w{name}", name=f"w{name}")
            # one DMA: [D, J] -> [p, ib, j] with p = d % 128, ib = d // 128
            nc.sync.dma_start(
                t[:].rearrange("p (ib j) -> p ib j", j=128),
                wd.rearrange("(ib p) j -> p ib j", p=128))
            w_sb[name] = t
        wo_sb = wpool.tile([J, D], BF16, tag="wo")
        nc.sync.dma_start(wo_sb[:], wo_d[:, :])

        bias = {}
        for name, bd in (("q", bq_d), ("k", bk_d), ("v", bv_d)):
            t = const.tile([J, 1], F32, tag=f"b{name}", name=f"b{name}")
            nc.sync.dma_start(t[:], bd.rearrange("(p o) -> p o", o=1))
            bias[name] = t

        # --- pipeline state ---------------------------------------------
        xt_tiles = {}    # (u, ib) -> [128, 512] bf16 x^T chunk tiles
        proj_t = {}      # (name, b) -> [128, 2048] bf16 q^T/k^T/v^T
        ve_t = {}        # (b, h) -> [128, 16*65] bf16 V natural + ones cols
        aot_t = {}       # u -> [128, 512] bf16 normalized attn-out^T
        qkv_ps = {}      # name -> pending psum tile during split emission

        def emit_xt_dma(b):
            # whole-batch x^T tiles: fewer, larger DMAs (4KB/partition each)
            for ib in range(8):
                t = xtp.tile([128, S], BF16, tag="xt", name="xt", bufs=16)
                nc.sync.dma_start(
                    t[:], xT_d[ib * 128:(ib + 1) * 128, b * S:(b + 1) * S])
                xt_tiles[(b, ib)] = t

        def qkv_unit(u, name, half):
            b, c = divmod(u, NCH)
            if half == 0 and name == "q" and c == 0:
                # new batch: allocate proj + ve tiles
                for nm in ("q", "k", "v"):
                    proj_t[(nm, b)] = projp.tile(
                        [128, S], BF16, tag=f"p{nm}", name=f"p{nm}", bufs=2)
                for h in range(HPC):
                    ve = vep.tile([128, NKB * 65], BF16, tag=f"ve{h}",
                                  name=f"ve{h}", bufs=2)
                    nc.vector.tensor_copy(
                        ve[:].rearrange("p (nb c) -> p nb c", c=65)[:, :, 64:65],
                        ones16[:].rearrange("p (a o) -> p a o", o=1),
                    )
                    ve_t[(b, h)] = ve
            if half == 0:
                ps = ps_sh.tile([128, CH], F32, tag="sh", name="qkv_ps", bufs=2)
                qkv_ps[name] = ps
                for ib in range(4):
                    nc.tensor.matmul(
                        ps[:], w_sb[name][:, ib * 128:(ib + 1) * 128],
                        xt_tiles[(b, ib)][:, c * CH:(c + 1) * CH],
                        start=(ib == 0), stop=False,
                    )
            else:
                ps = qkv_ps[name]
                for ib in range(4, 8):
                    nc.tensor.matmul(
                        ps[:], w_sb[name][:, ib * 128:(ib + 1) * 128],
                        xt_tiles[(b, ib)][:, c * CH:(c + 1) * CH],
                        start=False, stop=(ib == 7),
                    )
                # evac PSUM -> SBUF bf16 with per-partition bias add
                nc.vector.tensor_scalar(
                    proj_t[(name, b)][:, c * CH:(c + 1) * CH], ps[:],
                    bias[name][:], None, ADD,
                )
                if name == "v":
                    # V natural via PE transpose: one [128,128] transpose per
                    # k-block yields both heads' V columns
                    pv = proj_t[("v", b)]
                    for kb in range(4 * c, 4 * c + 4):
                        pst = ps_sh.tile([128, 256], BF16, tag="sh",
                                         name="vtp", bufs=2)
                        nc.tensor.transpose(
                            pst[:, 0:128],
                            pv[:, kb * 128:(kb + 1) * 128], ident[:],
                        )
                        for h in range(HPC):
                            nc.vector.tensor_copy(
                                ve_t[(b, h)][:, kb * 65: kb * 65 + 64],
                                pst[:, h * 64:(h + 1) * 64],
                            )

        def oproj_unit(u, ob):
            b, c = divmod(u, NCH)
            ps = ps_sh.tile([128, CH], F32, tag="sh", name="op_ps", bufs=2)
            nc.tensor.matmul(
                ps[:], wo_sb[:, ob * 128:(ob + 1) * 128], aot_t[u][:],
                start=True, stop=True,
            )
            stg = stgp.tile([128, CH], BF16, tag="stg", name="stg", bufs=4)
            if ob % 8 in (1, 3, 5):
                # balance PSUM evacuations across ACT and DVE
                nc.scalar.copy(stg[:], ps[:])
            else:
                nc.vector.tensor_copy(stg[:], ps[:])
            nc.sync.dma_start(
                out_d[ob * 128:(ob + 1) * 128,
                      b * S + c * CH: b * S + (c + 1) * CH],
                stg[:],
            )

        def emit_attention(u, fillers, op_fill):
            b, c = divmod(u, NCH)
            nkb = 4 * c + 4
            qt = proj_t[("q", b)]
            kt = proj_t[("k", b)]
            acc = [
                ps_av.tile([128, CH], F32, tag="av", name="acc0", bufs=2),
                ps_av.tile([128, CH], F32, tag="av", name="acc1", bufs=2),
            ]
            fill_i = 0

            def run_fillers(n):
                nonlocal fill_i
                for _ in range(n):
                    if fill_i < len(fillers):
                        fillers[fill_i]()
                        fill_i += 1
                    elif op_fill:
                        op_fill.popleft()()

            prev = None  # (p tile, kb, lo)
            for kb in range(nkb):
                lo = max(0, 128 * kb - CH * c)
                st = ps_sc.tile([128, 2 * CH], F32, tag="sc", name="st", bufs=2)
                for h in range(HPC):
                    nc.tensor.matmul(
                        st[:, h * CH + lo:(h + 1) * CH],
                        kt[h * 64:(h + 1) * 64, kb * 128:(kb + 1) * 128],
                        qt[h * 64:(h + 1) * 64, c * CH + lo:(c + 1) * CH],
                        start=True, stop=True,
                    )
                p = pp.tile([128, 2 * CH], BF16, tag="p", name="p", bufs=5)
                nc.scalar.activation(
                    p[:, lo:2 * CH], st[:, lo:2 * CH],
                    mybir.ActivationFunctionType.Exp, scale=0.125,
                )
                if kb >= 4 * c:  # diagonal block: mask lower triangle
                    for h in range(HPC):
                        # on the (otherwise idle) gpsimd engine
                        nc.gpsimd.tensor_mul(
                            p[:, h * CH + lo: h * CH + lo + 128],
                            p[:, h * CH + lo: h * CH + lo + 128],
                            mask[:],
                        )
                if prev is not None:
                    pprev, kbp, lop = prev
                    for h in range(HPC):
                        nc.tensor.matmul(
                            acc[h][0:65, lop:CH],
                            ve_t[(b, h)][:, kbp * 65: kbp * 65 + 65],
                            pprev[:, h * CH + lop:(h + 1) * CH],
                            start=(kbp == 0), stop=(kbp == nkb - 1),
                        )
                # cover the chunk-start exp latency with carried-over o_proj
                # fillers, then pace qkv fillers with a reserve for the tail
                if kb < 3 and op_fill:
                    op_fill.popleft()()
                run_fillers((len(fillers) - fill_i) // (nkb - kb + 3))
                prev = (p, kb, lo)
            pprev, kbp, lop = prev
            for h in range(HPC):
                nc.tensor.matmul(
                    acc[h][0:65, lop:CH],
                    ve_t[(b, h)][:, kbp * 65: kbp * 65 + 65],
                    pprev[:, h * CH + lop:(h + 1) * CH],
                    start=(kbp == 0), stop=(kbp == nkb - 1),
                )
            # normalize: row 64 of acc = softmax denominators
            aot = aotp.tile([128, CH], BF16, tag="aot", name="aot", bufs=3)
            aot_t[u] = aot
            for h in range(HPC):
                if FAST_RECIP:
                    # broadcast raw denominators d via the ones-matmul, then
                    # 1-pass NR approx recip on the full [64, CH] block (the
                    # single-row approx variant miscomputes on HW)
                    rec = smallp.tile([65, CH], F32R, tag="rec", name="rec",
                                      bufs=4)
                    nc.vector.tensor_copy(rec[64:65, :], acc[h][64:65, :])
                    bcp = ps_sh.tile([128, CH], F32, tag="sh", name="bcp",
                                     bufs=2)
                    nc.tensor.matmul(
                        bcp[0:64, :], onesM[64:65, :], rec[64:65, :],
                        start=True, stop=True,
                    )
                    bct = smallp.tile([64, CH], F32, tag="bct", name="bct",
                                      bufs=2)
                    nc.vector.tensor_copy(bct[:], bcp[0:64, :])
                    rr = smallp.tile([64, CH], F32, tag="rr", name="rr",
                                     bufs=2)
                    nc.vector.reciprocal_approx_fast(out=rr[:], in_=bct[:])
                else:
                    rec = smallp.tile([65, CH], F32R, tag="rec", name="rec",
                                      bufs=4)
                    with nc.allow_low_precision(reason="recip rounded to f32r"):
                        nc.vector.reciprocal(rec[64:65, :], acc[h][64:65, :])
                    bcp = ps_sh.tile([128, CH], F32, tag="sh", name="bcp",
                                     bufs=2)
                    nc.tensor.matmul(
                        bcp[0:64, :], onesM[64:65, :], rec[64:65, :],
                        start=True, stop=True,
                    )
                    rr = smallp.tile([64, CH], F32, tag="rr", name="rr",
                                     bufs=2)
                    nc.vector.tensor_copy(rr[:], bcp[0:64, :])
                if h == 0:
                    nc.vector.tensor_mul(aot[0:64, :], acc[h][0:64, :], rr[:])
                else:
                    tmp = smallp.tile([64, CH], BF16, tag="tmp", name="tmp",
                                      bufs=2)
                    nc.vector.tensor_mul(tmp[:], acc[h][0:64, :], rr[:])
                    # partition shift 0-63 -> 64-127 via SBUF->SBUF DMA
                    nc.sync.dma_start(aot[64:128, :], tmp[:])
                run_fillers(2)
            run_fillers(len(fillers))

        # --- steps --------------------------------------------------------
        from collections import deque

        op_fill = deque()
        emit_xt_dma(0)
        for s in range(NU + 1):
            if s % NCH == 0 and s // NCH + 1 < B:
                emit_xt_dma(s // NCH + 1)
            fillers = []
            if s < NU:
                for name in ("q", "k", "v"):
                    for half in range(2):
                        fillers.append(
                            lambda u=s, n=name, hf=half: qkv_unit(u, n, hf))
            if 2 <= s:
                for ob in range(8):
                    op_fill.append(lambda u=s - 2, o=ob: oproj_unit(u, o))
            if 1 <= s <= NU:
                emit_attention(s - 1, fillers, op_fill)
            else:
                for f in fillers:
                    f()
        # tail: o_proj for the last two chunks
        for u in (NU - 1,):
            for ob in range(8):
                op_fill.append(lambda u=u, o=ob: oproj_unit(u, o))
        while op_fill:
            op_fill.popleft()()


_NC_CACHE = None


def _get_nc():
    global _NC_CACHE
    if _NC_CACHE is None:
        _NC_CACHE = build_kernel()
    return _NC_CACHE


def _to_bf16(a):
    import ml_dtypes
    return np.asarray(a).astype(ml_dtypes.bfloat16)


def make_in_maps(inputs):
    x = np.asarray(inputs["hidden_states"], np.float32).reshape(BS, D)
    xT = np.ascontiguousarray(_to_bf16(x).T)  # [D, BS] bf16
    Wq = _to_bf16(inputs["Wq"])
    Wk = _to_bf16(inputs["Wk"])
    Wv = _to_bf16(inputs["Wv"])
    Wo = _to_bf16(inputs["Wo"])
    bq = np.asarray(inputs["bq"], np.float32)
    bk = np.asarray(inputs["bk"], np.float32)
    bv = np.asarray(inputs["bv"], np.float32)

    in_maps = []
    for c in range(N_CORES):
        js = slice(c * J, (c + 1) * J)
        in_maps.append({
            "xT": xT,
            "wq": np.ascontiguousarray(Wq[:, js]),
            "wk": np.ascontiguousarray(Wk[:, js]),
            "wv": np.ascontiguousarray(Wv[:, js]),
            "wo": np.ascontiguousarray(Wo[js, :]),
            "bq": np.ascontiguousarray(bq[js]),
            "bk": np.ascontiguousarray(bk[js]),
            "bv": np.ascontiguousarray(bv[js]),
        })
    return in_maps


def gather_out(results, bo):
    out_t = np.zeros((D, BS), np.float32)
    for c in range(N_CORES):
        out_t += results[c]["out_t"].astype(np.float32)
    out = out_t.T + np.asarray(bo, np.float32)[None, :]
    return out.reshape(B, S, D).astype(np.float32)


def kernel(**inputs) -> np.ndarray:
    nc = _get_nc()
    in_maps = make_in_maps(inputs)
    res = run_bass_kernel_spmd(nc, in_maps, core_ids=list(range(N_CORES)))
    return gather_out(res.results, inputs["bo"])


if __name__ == "__main__":
    rng = np.random.default_rng(0)
    ins = {
        "hidden_states": rng.standard_normal((B, S, D), np.float32),
        "Wq": rng.standard_normal((D, D), np.float32) * 0.02,
        "bq": np.zeros(D, np.float32),
        "Wk": rng.standard_normal((D, D), np.float32) * 0.02,
        "bk": np.zeros(D, np.float32),
        "Wv": rng.standard_normal((D, D), np.float32) * 0.02,
        "bv": np.zeros(D, np.float32),
        "Wo": rng.standard_normal((D, D), np.float32) * 0.02,
        "bo": np.zeros(D, np.float32),
    }
    out = kernel(**ins)
    print("out", out.shape, out.dtype, float(np.abs(out).mean()))
